# revision 1
# baseline (speedup 1.0000x reference)
"""Decoder-only transformer (GPT-style, post-LN) forward pass on 8 Trainium2 cores.

Sharding: tokens (batch*seq) are block-sharded 8 ways for the embedding and the
4 transformer layers (core c owns batch c//4, seq chunk c%4 of 512 tokens).
K/V are all-gathered per layer within each batch's 4-core group. After the
final layernorm the hidden states are all-gathered across all 8 cores and the
LM head is vocab-sharded (6400 padded columns per core).

Everything is computed on device; the host only slices/reshapes inputs,
precomputes the (constant) positional-encoding table and causal masks, and
concatenates the 8 logits shards.
"""

import math
import os

import numpy as np
import ml_dtypes

import concourse.bass as bass
import concourse.bacc as bacc
import concourse.mybir as mybir
import concourse.tile as tile
from concourse.bass_utils import run_bass_kernel_spmd
from concourse.masks import make_identity

# model dims (hardcoded per problem spec)
V, S, D, NL, H = 50257, 2048, 768, 4, 12
HD, DF, B = 64, 3072, 2
NC = 8          # cores
CH = 512        # tokens per core
QT = 4          # 128-token tiles per core
DT = 6          # 128-wide d tiles
FT = 24         # 128-wide dff tiles
VS = 6400       # padded vocab shard per core (8*6400 = 51200 >= 50257)
ESH = 6283      # embedding-table rows per core (8*6283 = 50264 >= 50257)
RANKS = 4       # cores per batch group

F32 = mybir.dt.float32
F32R = mybir.dt.float32r
BF16 = mybir.dt.bfloat16
I32 = mybir.dt.int32
AX = mybir.AxisListType.X
OP = mybir.AluOpType
AF = mybir.ActivationFunctionType
P = 128

_CACHE = {}


def build():
    nc = bacc.Bacc(None, target_bir_lowering=False, num_devices=NC)

    # ---- kernel I/O ----
    ids = nc.dram_tensor("ids", [P, QT], I32, kind="ExternalInput")
    pe_in = nc.dram_tensor("pe", [P, QT, D], F32, kind="ExternalInput")
    masks_in = nc.dram_tensor("masks", [P, 16, CH], BF16, kind="ExternalInput")
    tok_emb = nc.dram_tensor("tok_emb", [V, D], F32, kind="ExternalInput")
    wq_d = nc.dram_tensor("wq", [NL, D, D], F32R, kind="ExternalInput")
    wk_d = nc.dram_tensor("wk", [NL, D, D], F32R, kind="ExternalInput")
    wv_d = nc.dram_tensor("wv", [NL, D, D], F32R, kind="ExternalInput")
    wo_d = nc.dram_tensor("wo", [NL, D, D], F32R, kind="ExternalInput")
    w1_d = nc.dram_tensor("w1", [NL, D, DF], F32R, kind="ExternalInput")
    w2_d = nc.dram_tensor("w2", [NL, DF, D], F32R, kind="ExternalInput")
    b1_d = nc.dram_tensor("b1", [NL, DF], F32, kind="ExternalInput")
    b2_d = nc.dram_tensor("b2", [NL, D], F32R, kind="ExternalInput")
    ln1g_d = nc.dram_tensor("ln1_g", [NL, D], F32R, kind="ExternalInput")
    ln1b_d = nc.dram_tensor("ln1_b", [NL, D], F32R, kind="ExternalInput")
    ln2g_d = nc.dram_tensor("ln2_g", [NL, D], F32R, kind="ExternalInput")
    ln2b_d = nc.dram_tensor("ln2_b", [NL, D], F32R, kind="ExternalInput")
    lnfg_d = nc.dram_tensor("lnf_g", [1, D], F32R, kind="ExternalInput")
    lnfb_d = nc.dram_tensor("lnf_b", [1, D], F32R, kind="ExternalInput")
    lmw_d = nc.dram_tensor("lm_w", [D, VS], F32R, kind="ExternalInput")
    lmb_d = nc.dram_tensor("lm_b", [1, VS], F32R, kind="ExternalInput")
    ones_d = nc.dram_tensor("c_ones", [1, P], F32R, kind="ExternalInput")
    logits = nc.dram_tensor("logits", [NC * CH, VS], F32, kind="ExternalOutput")

    g4 = [[0, 1, 2, 3], [4, 5, 6, 7]]
    g8 = [list(range(NC))]

    with tile.TileContext(nc) as tc:
        with (
            tc.tile_pool(name="pers", bufs=1) as pers,
            tc.tile_pool(name="dram", bufs=1, space="DRAM") as dram,
        ):
            h = pers.tile([P, QT, D], F32, name="h_res")
            ones_sb = pers.tile([1, P], F32R, name="ones_sb")
            nc.sync.dma_start(ones_sb[:], ones_d[:])
            id32 = pers.tile([P, P], F32, name="id32")
            make_identity(nc, id32[:])
            idb = pers.tile([P, P], BF16, name="idb")
            make_identity(nc, idb[:])

            # ---------- embedding: gather + positional encoding ----------
            with tc.tile_pool(name="embp", bufs=1) as ep:
                ids_sb = ep.tile([P, QT], I32)
                nc.sync.dma_start(ids_sb[:], ids[:])
                pe_sb = ep.tile([P, QT, D], F32)
                nc.sync.dma_start(pe_sb[:], pe_in[:])
                for qt in range(QT):
                    emb = ep.tile([P, D], F32, tag="emb", bufs=2)
                    nc.gpsimd.indirect_dma_start(
                        out=emb[:],
                        out_offset=None,
                        in_=tok_emb[:],
                        in_offset=bass.IndirectOffsetOnAxis(ap=ids_sb[:, qt : qt + 1], axis=0),
                    )
                    nc.vector.tensor_tensor(h[:, qt, :], emb[:], pe_sb[:, qt, :], OP.add)

            # ---------- transformer layers ----------
            with (
                tc.tile_pool(name="wk", bufs=1) as wk,
                tc.tile_pool(name="psb", bufs=1, space="PSUM") as psb,
            ):
                lnp_g = wk.tile([P, D], F32, tag="lnpg", name="lnp_g")
                lnp_b = wk.tile([P, D], F32, tag="lnpb", name="lnp_b")
                scr = wk.tile([P, D], F32, tag="scr", name="scr")
                prow = wk.tile([1, D], F32R, tag="prow", name="prow")

                def bcast_row(dst, row_dram_ap):
                    """dst[p, :] = row for all p (via K=1 matmul)."""
                    nc.sync.dma_start(prow[:], row_dram_ap)
                    pb = psb.tile([P, D], F32, tag="big", bufs=2, name="pb_bcast")
                    nc.tensor.matmul(pb[:, 0:512], ones_sb[:], prow[:, 0:512], start=True, stop=True)
                    nc.tensor.matmul(pb[:, 512:D], ones_sb[:], prow[:, 512:D], start=True, stop=True)
                    nc.vector.tensor_copy(out=dst[:], in_=pb[:])

                def layernorm(g_row, b_row):
                    """in-place LN over the feature axis of h."""
                    bcast_row(lnp_g, g_row)
                    bcast_row(lnp_b, b_row)
                    for qt in range(QT):
                        x = h[:, qt, :]
                        ssum = wk.tile([P, 1], F32, tag="st1", name="ssum")
                        nc.vector.tensor_reduce(out=ssum[:], in_=x, axis=AX, op=OP.add)
                        ssq = wk.tile([P, 1], F32, tag="st2", name="ssq")
                        nc.scalar.activation(scr[:], x, AF.Square, accum_out=ssq[:])
                        mean = wk.tile([P, 1], F32, tag="st3", name="mean")
                        nc.vector.tensor_scalar_mul(mean[:], ssum[:], 1.0 / D)
                        bias_t = wk.tile([P, 1], F32, tag="st4", name="bias_t")
                        nc.vector.tensor_tensor(bias_t[:], mean[:], mean[:], OP.mult)
                        nc.vector.tensor_scalar(bias_t[:], bias_t[:], -1.0, 1e-5, OP.mult, OP.add)
                        sstd = wk.tile([P, 1], F32, tag="st5", name="sstd")
                        nc.scalar.activation(sstd[:], ssq[:], AF.Sqrt, bias=bias_t[:], scale=1.0 / D)
                        rstd = wk.tile([P, 1], F32, tag="st6", name="rstd")
                        nc.vector.reciprocal(rstd[:], sstd[:])
                        nc.vector.tensor_scalar(scr[:], x, mean[:], rstd[:], OP.subtract, OP.mult)
                        nc.vector.tensor_tensor(scr[:], scr[:], lnp_g[:], OP.mult)
                        nc.vector.tensor_tensor(h[:, qt, :], scr[:], lnp_b[:], OP.add)

                def transpose_h(src_ap_fn, dst, ident=None, pdt=F32):
                    """dst[:, dt, qt*128:...] = transpose of 128x128 blocks of token-major src."""
                    if ident is None:
                        ident = id32
                    for qt in range(QT):
                        for dt in range(DT):
                            pt = psb.tile([P, 512], pdt, tag="sc", bufs=2, name="pt_tr")
                            nc.tensor.transpose(pt[:, :P], src_ap_fn(qt, dt), ident[:])
                            nc.vector.tensor_copy(out=dst[:, dt, qt * P : (qt + 1) * P], in_=pt[:, :P])

                for l in range(NL if not os.environ.get("TRN_SKIP_LAYERS") else 0):
                    with nc.named_scope(f"layer{l}"):
                        # --- h^T (f32r) for all projections ---
                        hT = wk.tile([P, DT, CH], F32R, tag="t6", bufs=2, name=f"hT_{l}")
                        transpose_h(lambda qt, dt: h[:, qt, dt * P : (dt + 1) * P], hT)

                        # --- K^T = (h @ wk)^T, scaled by 1/sqrt(hd) ---
                        kT_w = wk.tile([P, DT, CH], F32R, tag="t6", bufs=2, name=f"kT_{l}")
                        for od in range(DT):
                            wqs = wk.tile([P, DT, P], F32R, tag="wqs", bufs=2, name="wk_c")
                            nc.sync.dma_start(
                                wqs[:],
                                wk_d[l].rearrange("(o p) f -> p o f", p=P)[:, :, od * P : (od + 1) * P],
                            )
                            ps = psb.tile([P, 512], F32, tag="sc", bufs=2, name="ps_k")
                            for kt in range(DT):
                                nc.tensor.matmul(
                                    ps[:], wqs[:, kt, :], hT[:, kt, :], start=(kt == 0), stop=(kt == DT - 1)
                                )
                            nc.vector.tensor_scalar_mul(kT_w[:, od, :], ps[:], HD ** (-0.5))
                        kt_in = dram.tile([D, CH], F32R, name=f"kt_in{l}")
                        nc.sync.dma_start(kt_in.rearrange("(o p) f -> p o f", p=P), kT_w[:])

                        # --- V = h @ wv (token-major, ones column per head) ---
                        wvf = wk.tile([P, DT, D], F32R, tag="wvf", name="wv_f")
                        nc.sync.dma_start(wvf[:], wv_d[l].rearrange("(o p) f -> p o f", p=P))
                        v_w = wk.tile([P, QT, H, HD + 1], BF16, tag="vw", name="v_w")
                        nc.vector.memset(v_w[:, :, :, HD], 1.0)
                        for qt in range(QT):
                            pv = psb.tile([P, D], F32, tag="big", bufs=2, name="ps_v")
                            for kt in range(DT):
                                nc.tensor.matmul(
                                    pv[:, 0:512],
                                    hT[:, kt, qt * P : (qt + 1) * P],
                                    wvf[:, kt, 0:512],
                                    start=(kt == 0),
                                    stop=(kt == DT - 1),
                                )
                                nc.tensor.matmul(
                                    pv[:, 512:D],
                                    hT[:, kt, qt * P : (qt + 1) * P],
                                    wvf[:, kt, 512:D],
                                    start=(kt == 0),
                                    stop=(kt == DT - 1),
                                )
                            nc.vector.tensor_copy(
                                out=v_w[:, qt, :, 0:HD],
                                in_=pv[:].rearrange("p (h e) -> p h e", e=HD),
                            )
                        v_in = dram.tile([CH, H * (HD + 1)], BF16, name=f"v_in{l}")
                        nc.sync.dma_start(
                            v_in.rearrange("(q p) (h e) -> p q h e", p=P, e=HD + 1), v_w[:]
                        )

                        # --- all-gather K^T and V within each batch group ---
                        kt_ag = dram.tile([RANKS * D, CH], F32R, name=f"kt_ag{l}")
                        nc.gpsimd.collective_compute(
                            "AllGather", OP.bypass, replica_groups=g4,
                            ins=[kt_in[:].opt()], outs=[kt_ag[:].opt()],
                        )
                        v_ag = dram.tile([RANKS * CH, H * (HD + 1)], BF16, name=f"v_ag{l}")
                        nc.gpsimd.collective_compute(
                            "AllGather", OP.bypass, replica_groups=g4,
                            ins=[v_in[:].opt()], outs=[v_ag[:].opt()],
                        )

                        # --- Q^T = (h @ wq)^T ---
                        qT = wk.tile([P, DT, CH], F32R, tag="q6", name=f"qT_{l}")
                        for od in range(DT):
                            wqs = wk.tile([P, DT, P], F32R, tag="wqs", bufs=2, name="wq_c")
                            nc.sync.dma_start(
                                wqs[:],
                                wq_d[l].rearrange("(o p) f -> p o f", p=P)[:, :, od * P : (od + 1) * P],
                            )
                            ps = psb.tile([P, 512], F32, tag="sc", bufs=2, name="ps_q")
                            for kt in range(DT):
                                nc.tensor.matmul(
                                    ps[:], wqs[:, kt, :], hT[:, kt, :], start=(kt == 0), stop=(kt == DT - 1)
                                )
                            nc.vector.tensor_copy(out=qT[:, od, :], in_=ps[:])

                        # --- attention ---
                        vag_sb = wk.tile([P, 16, H * (HD + 1)], BF16, tag="vag", name=f"vag_{l}")
                        nc.sync.dma_start(vag_sb[:], v_ag.rearrange("(o p) f -> p o f", p=P))
                        o_acc = wk.tile([P, QT, H, HD + 1], F32, tag="oacc", name=f"oacc_{l}")
                        for r in range(RANKS):
                            ktc = wk.tile([P, DT, CH], F32R, tag="kts", bufs=2, name="ktc")
                            nc.sync.dma_start(
                                ktc[:], kt_ag[D * r : D * (r + 1), :].rearrange("(o p) f -> p o f", p=P)
                            )
                            mks = wk.tile([P, QT, CH], BF16, tag="mks", bufs=1, name="mks")
                            nc.sync.dma_start(mks[:], masks_in[:, r * 4 : (r + 1) * 4, :])
                            for hh in range(H):
                                pb_ = (hh % 2) * 64
                                od = hh // 2
                                es = [None] * 4
                                for kt in range(4):
                                    ps_s = psb.tile([P, 512], F32, tag="sc", bufs=2, name="ps_s")
                                    nc.tensor.matmul(
                                        ps_s[:],
                                        ktc[pb_ : pb_ + 64, od, kt * P : (kt + 1) * P],
                                        qT[pb_ : pb_ + 64, od, :],
                                        start=True,
                                        stop=True,
                                    )
                                    e = wk.tile([P, CH], BF16, tag="es", bufs=4, name="es")
                                    nc.scalar.activation(e[:], ps_s[:], AF.Exp)
                                    nc.vector.tensor_tensor(e[:], e[:], mks[:, kt, :], OP.mult)
                                    es[kt] = e
                                for qt in range(QT):
                                    pav = psb.tile([P, HD + 1], F32, tag="avq", bufs=2, name="pav")
                                    for kt in range(4):
                                        nc.tensor.matmul(
                                            pav[:],
                                            es[kt][:, qt * P : (qt + 1) * P],
                                            vag_sb[:, r * 4 + kt, hh * (HD + 1) : (hh + 1) * (HD + 1)],
                                            start=(kt == 0),
                                            stop=(kt == 3),
                                        )
                                    if r == 0:
                                        nc.vector.tensor_copy(out=o_acc[:, qt, hh, :], in_=pav[:])
                                    else:
                                        nc.vector.tensor_tensor(
                                            o_acc[:, qt, hh, :], o_acc[:, qt, hh, :], pav[:], OP.add
                                        )
                        # normalize by the ones-column sums into a contiguous tile
                        o_nrm = wk.tile([P, QT, D], BF16, tag="onrm", name=f"onrm_{l}")
                        for qt in range(QT):
                            rec = wk.tile([P, H], F32, tag="rec", name="rec")
                            nc.vector.reciprocal(rec[:], o_acc[:, qt, :, HD])
                            nc.vector.tensor_tensor(
                                o_nrm[:, qt, :].rearrange("p (h e) -> p h e", e=HD),
                                o_acc[:, qt, :, 0:HD],
                                rec[:, :, None].to_broadcast([P, H, HD]),
                                OP.mult,
                            )

                        # --- O^T then mha = O @ wo, residual, LN1 ---
                        oT = wk.tile([P, DT, CH], F32R, tag="t6", bufs=2, name=f"oT_{l}")
                        transpose_h(
                            lambda qt, dt: o_nrm[:, qt, dt * P : (dt + 1) * P], oT, ident=idb, pdt=BF16
                        )
                        wof = wk.tile([P, DT, D], F32R, tag="wvf", name="wo_f")
                        nc.sync.dma_start(wof[:], wo_d[l].rearrange("(o p) f -> p o f", p=P))
                        for qt in range(QT):
                            pm = psb.tile([P, D], F32, tag="big", bufs=2, name="ps_wo")
                            for kt in range(DT):
                                nc.tensor.matmul(
                                    pm[:, 0:512],
                                    oT[:, kt, qt * P : (qt + 1) * P],
                                    wof[:, kt, 0:512],
                                    start=(kt == 0),
                                    stop=(kt == DT - 1),
                                )
                                nc.tensor.matmul(
                                    pm[:, 512:D],
                                    oT[:, kt, qt * P : (qt + 1) * P],
                                    wof[:, kt, 512:D],
                                    start=(kt == 0),
                                    stop=(kt == DT - 1),
                                )
                            nc.vector.tensor_tensor(h[:, qt, :], h[:, qt, :], pm[:], OP.add)
                        layernorm(ln1g_d[l : l + 1, :], ln1b_d[l : l + 1, :])

                        # --- FFN ---
                        hT2 = wk.tile([P, DT, CH], F32R, tag="t6", bufs=2, name=f"hT2_{l}")
                        transpose_h(lambda qt, dt: h[:, qt, dt * P : (dt + 1) * P], hT2)
                        b1_sb = wk.tile([P, FT], F32, tag="b1s", name="b1_sb")
                        nc.sync.dma_start(b1_sb[:], b1_d[l : l + 1, :].rearrange("a (o p) -> p (a o)", p=P))
                        b2_sb = wk.tile([1, D], F32R, tag="b2s", name="b2_sb")
                        nc.sync.dma_start(b2_sb[:], b2_d[l : l + 1, :])
                        for qp in range(2):
                            f1c = wk.tile([P, FT, 256], F32R, tag="f1c", name="f1c")
                            for df in range(FT):
                                w1c = wk.tile([P, DT, P], F32R, tag="w1s", bufs=2, name="w1c")
                                nc.sync.dma_start(
                                    w1c[:],
                                    w1_d[l].rearrange("(o p) f -> p o f", p=P)[:, :, df * P : (df + 1) * P],
                                )
                                pf1 = psb.tile([P, 512], F32, tag="sc", bufs=2, name="ps_f1")
                                for kt in range(DT):
                                    nc.tensor.matmul(
                                        pf1[:, 0:256],
                                        w1c[:, kt, :],
                                        hT2[:, kt, qp * 256 : (qp + 1) * 256],
                                        start=(kt == 0),
                                        stop=(kt == DT - 1),
                                    )
                                nc.scalar.activation(
                                    f1c[:, df, :], pf1[:, 0:256], AF.Relu, bias=b1_sb[:, df : df + 1]
                                )
                            pf2 = [None, None]
                            for qtl in range(2):
                                pf2[qtl] = psb.tile([P, D], F32, tag="big", bufs=2, name="ps_f2")
                                nc.tensor.matmul(pf2[qtl][:, 0:512], ones_sb[:], b2_sb[:, 0:512], start=True, stop=False)
                                nc.tensor.matmul(pf2[qtl][:, 512:D], ones_sb[:], b2_sb[:, 512:D], start=True, stop=False)
                            for df in range(FT):
                                w2c = wk.tile([P, D], F32R, tag="w2s", bufs=2, name="w2c")
                                nc.sync.dma_start(
                                    w2c[:], w2_d[l].rearrange("(o p) f -> p o f", p=P)[:, df, :]
                                )
                                for qtl in range(2):
                                    nc.tensor.matmul(
                                        pf2[qtl][:, 0:512],
                                        f1c[:, df, qtl * P : (qtl + 1) * P],
                                        w2c[:, 0:512],
                                        start=False,
                                        stop=(df == FT - 1),
                                    )
                                    nc.tensor.matmul(
                                        pf2[qtl][:, 512:D],
                                        f1c[:, df, qtl * P : (qtl + 1) * P],
                                        w2c[:, 512:D],
                                        start=False,
                                        stop=(df == FT - 1),
                                    )
                            for qtl in range(2):
                                qt = qp * 2 + qtl
                                nc.vector.tensor_tensor(h[:, qt, :], h[:, qt, :], pf2[qtl][:], OP.add)
                        layernorm(ln2g_d[l : l + 1, :], ln2b_d[l : l + 1, :])

                # ---------- final LN, all-gather h^T across 8 cores ----------
                with nc.named_scope("final"):
                    layernorm(lnfg_d[:], lnfb_d[:])
                    hTf = wk.tile([P, DT, CH], F32R, tag="t6", bufs=2, name="hTf")
                    transpose_h(lambda qt, dt: h[:, qt, dt * P : (dt + 1) * P], hTf)
                    hT_in = dram.tile([D, CH], F32R, name="hT_in")
                    nc.sync.dma_start(hT_in.rearrange("(o p) f -> p o f", p=P), hTf[:])
                    hT_ag = dram.tile([NC * D, CH], F32R, name="hT_ag", addr_space="Shared")
                    nc.gpsimd.collective_compute(
                        "AllGather", OP.bypass, replica_groups=g8,
                        ins=[hT_in[:].opt()], outs=[hT_ag[:].opt()],
                    )

            # ---------- LM head (vocab-sharded) ----------
            if not os.environ.get("TRN_SKIP_LM"):
                with (
                    tc.tile_pool(name="lmp", bufs=1) as lmp,
                    tc.tile_pool(name="pslm", bufs=1, space="PSUM") as pslm,
                    nc.named_scope("lmhead"),
                ):
                    htag = lmp.tile([P, NC * DT, CH], F32R, name="htag")
                    nc.sync.dma_start(htag[:], hT_ag.rearrange("(o p) f -> p o f", p=P))
                    lmb_sb = lmp.tile([1, VS], F32R, name="lmb_sb")
                    nc.sync.dma_start(lmb_sb[:], lmb_d[:])
                    nch = [(i * 512, 512) for i in range(12)] + [(12 * 512, 256)]
                    for n0, nsz in nch:
                        lmw_c = lmp.tile([P, DT, 512], F32R, tag="lmw", bufs=3, name="lmw_c")
                        nc.sync.dma_start(
                            lmw_c[:, :, :nsz],
                            lmw_d.rearrange("(o p) f -> p o f", p=P)[:, :, n0 : n0 + nsz],
                        )
                        for m in range(NC * QT):
                            po = pslm.tile([P, 512], F32, tag="lmo", bufs=4, name="ps_lm")
                            nc.tensor.matmul(po[:, :nsz], ones_sb[:], lmb_sb[:, n0 : n0 + nsz], start=True, stop=False)
                            rr, qt = m // QT, m % QT
                            for dt in range(DT):
                                nc.tensor.matmul(
                                    po[:, :nsz],
                                    htag[:, DT * rr + dt, qt * P : (qt + 1) * P],
                                    lmw_c[:, dt, :nsz],
                                    start=False,
                                    stop=(dt == DT - 1),
                                )
                            osb = lmp.tile([P, 512], F32, tag="osb", bufs=4, name="o_sb")
                            nc.vector.tensor_copy(out=osb[:, :nsz], in_=po[:, :nsz])
                            nc.sync.dma_start(
                                logits[rr * CH + qt * P : rr * CH + (qt + 1) * P, n0 : n0 + nsz],
                                osb[:, :nsz],
                            )

    return _finish(nc)


def _finish(nc):
    nc.compile()
    return nc


def _pe_table():
    pos = np.arange(S, dtype=np.float32)[:, None]
    div = np.exp(np.arange(0, D, 2, dtype=np.float32) * (-math.log(10000.0) / D))
    pe = np.zeros((S, D), dtype=np.float32)
    pe[:, 0::2] = np.sin(pos * div)
    pe[:, 1::2] = np.cos(pos * div)
    return pe


def kernel(**inputs):
    if "nc" not in _CACHE:
        _CACHE["nc"] = build()
    nc = _CACHE["nc"]

    x = np.asarray(inputs["x"])
    f32 = lambda a: np.ascontiguousarray(np.asarray(a), dtype=np.float32)
    all_ids = x.reshape(-1).astype(np.int64)  # [4096] rank-major token order
    # stack per-head projections into [D, H*HD]
    wq = f32(inputs["wq"]).transpose(0, 2, 1, 3).reshape(NL, D, D)
    wk_ = f32(inputs["wk"]).transpose(0, 2, 1, 3).reshape(NL, D, D)
    wv = f32(inputs["wv"]).transpose(0, 2, 1, 3).reshape(NL, D, D)
    wo = f32(inputs["wo"])
    w1 = f32(inputs["w1"])
    w2 = f32(inputs["w2"])
    tok = f32(inputs["tok_emb"])
    lm_w = f32(inputs["lm_w"])
    lm_b = f32(inputs["lm_b"])
    pe = _pe_table()
    ones_c = np.ones((1, P), dtype=np.float32)

    common = {
        "tok_emb": tok,
        "wq": wq, "wk": wk_, "wv": wv, "wo": wo,
        "w1": w1, "w2": w2,
        "b1": f32(inputs["b1"]), "b2": f32(inputs["b2"]),
        "ln1_g": f32(inputs["ln1_g"]), "ln1_b": f32(inputs["ln1_b"]),
        "ln2_g": f32(inputs["ln2_g"]), "ln2_b": f32(inputs["ln2_b"]),
        "lnf_g": f32(inputs["lnf_g"]).reshape(1, D),
        "lnf_b": f32(inputs["lnf_b"]).reshape(1, D),
        "c_ones": ones_c,
    }

    lmw_pad = np.zeros((D, NC * VS), dtype=np.float32)
    lmw_pad[:, :V] = lm_w
    lmb_pad = np.zeros((NC * VS,), dtype=np.float32)
    lmb_pad[:V] = lm_b

    in_maps = []
    for c in range(NC):
        b, j = c // RANKS, c % RANKS
        toks = x[b, j * CH : (j + 1) * CH].astype(np.int32)  # [512]
        ids_c = toks.reshape(QT, P).T.copy()  # [128, 4]
        pe_c = pe[j * CH : (j + 1) * CH].reshape(QT, P, D).transpose(1, 0, 2).copy()
        kidx = np.arange(16 * P).reshape(16, P)  # [gkt, p] -> global k
        qidx = j * CH + np.arange(CH)  # [f] -> global q
        m = (kidx[None, :, :, None] <= qidx[None, None, None, :])  # [1,16,128,512]
        masks_c = m[0].transpose(1, 0, 2).astype(ml_dtypes.bfloat16)  # [128,16,512]
        in_maps.append({
            **common,
            "ids": ids_c,
            "pe": pe_c,
            "masks": np.ascontiguousarray(masks_c),
            "lm_w": np.ascontiguousarray(lmw_pad[:, c * VS : (c + 1) * VS]),
            "lm_b": np.ascontiguousarray(lmb_pad[c * VS : (c + 1) * VS]).reshape(1, VS),
        })

    import os

    trace = bool(os.environ.get("TRN_KERNEL_TRACE"))
    res = run_bass_kernel_spmd(nc, in_maps, core_ids=list(range(NC)), trace=trace)
    _CACHE["last_result"] = res
    _CACHE["last_in_maps"] = in_maps
    out = np.concatenate([res.results[c]["logits"] for c in range(NC)], axis=1)
    return out[:, :V].reshape(B, S, V).astype(np.float32)


if __name__ == "__main__":
    import time

    t0 = time.time()
    nc = build()
    print(f"build ok: {time.time() - t0:.1f}s")

